# revision 64
# baseline (speedup 1.0000x reference)
"""Trainium2 Bass kernel for MessageControlGraphAttentionLayer (sparse form).

Key insight: mask[j,k]=1 only when j,k lie in the same zero-run of
boundary (plus the diagonal), so only ~0.8% of (j,k) pairs carry a
computed logit. For masked pairs exp(0)=1, so softmax collapses:
  Z[j,h]   = (T - L_j) + sum_{k in seg(j)} exp(l[j,k,h])
  x1[j,:,h]= ( (A - S_seg(j)) + sum_{k in seg(j)} exp(l)*x[k] ) / Z
with A = sum_k x[k,:] and S_seg = sum_{k in seg} x[k,:]. A - S comes
from one matmul with (1 - mask); the kept pairs live on a compact
pair axis p (host-gathered columns XJg/XKg, selector matrices JSEL).

Per core (batch b=c//2, rows j in [128*(c%2), ...+128)):
  PP[d,p] = XJg*XKg (DVE) -> mm1 (4 matmuls, bf16) -> tanh+b1 (ACT)
  mm2: l[p,h] via lhsT=a-chunk, rhs=W2 (N=8) -> exp -> Z via JSEL
  zi = 1/((T-L)+Z) -> zin gather via JSELT -> en = exp*zin
  JSELh[p,j] = en[:,h]*JSEL (per h); x1T[d,j] += Xp^T @ JSELh
  pseudo-chunk: (A-S)^T with diag(zi_h) rhs handles the masked mass.
  mm4: y = Wp^T x1 + Wn^T x + biases; BN stats AllReduce; affine+selu.

All weights/x in bf16 (validated ~4e-3 end-to-end rel err).
"""

import sys

if "/opt/trn_rl_repo" not in sys.path:
    sys.path.insert(0, "/opt/trn_rl_repo")

import numpy as np
import ml_dtypes

B, T, D, O, H = 4, 256, 256, 256, 8
P = 128
NCORES = 8
J = 128  # rows per core
BN_EPS = 1e-5
SELU_LAM = 1.0507009873554805
SELU_ALPHA = 1.6732632423543772

BF = ml_dtypes.bfloat16

_CACHE = {}


def _message_control_mask_np(boundary):
    Bb, Tt = boundary.shape
    s = np.cumsum(boundary.astype(np.int64), axis=1)
    spad = np.concatenate([np.zeros((Bb, 1), np.int64), s], axis=1)
    idx = np.arange(Tt)
    jj, kk = np.meshgrid(idx, idx, indexing="ij")
    rng_sum = spad[:, np.maximum(jj, kk) + 1] - spad[:, np.minimum(jj, kk)]
    mask = rng_sum == 0
    mask = mask | np.eye(Tt, dtype=bool)[None]
    return mask.astype(np.float32)


def _seg_of(brow):
    """Per-row (k0, L): the maximal zero-run containing the row, or the
    singleton (j, 1) for boundary rows (diagonal-only)."""
    seg = np.zeros((T, 2), np.int64)
    i = 0
    while i < T:
        if brow[i] == 0:
            j = i
            while j < T and brow[j] == 0:
                j += 1
            seg[i:j, 0] = i
            seg[i:j, 1] = j - i
            i = j
        else:
            seg[i] = (i, 1)
            i += 1
    return seg


def _build_module(with_collective=True, reps=1, nch=None):
    from concourse import bacc, tile
    import concourse.mybir as mybir
    from concourse.masks import make_identity

    if nch is None:
        nch = _CACHE.get("nch", 3)
    NCH = nch
    PAD = NCH * P

    f32 = mybir.dt.float32
    bf16 = mybir.dt.bfloat16
    AF = mybir.ActivationFunctionType
    ALU = mybir.AluOpType

    nc = bacc.Bacc("TRN2", target_bir_lowering=False, debug=False,
                   num_devices=NCORES)

    # packed inputs: 4 DMAs instead of 18 (HWDGE queue is 625ns/DMA)
    # pz   f32 [P, 9]: pvec (8) + zc (1)
    # early bf16 [P, 2*PAD + 2*PAD + 512 + 16]: xjg, xkg, w1l, w2c
    # mid  bf16 [P, NCH*128*2 + 256 + 512 + NCH*256 + 256 + 512]:
    #            jsel, jselt, notm, xk, xp, xtl, wnl
    # wpl  bf16 [P, 4096]
    NM = 2 * NCH * P + 256 + 512 + NCH * 2 * P + 256 + 512
    pz_d = nc.dram_tensor("pz", [P, 9], f32, kind="ExternalInput")
    xjk_d = nc.dram_tensor("xjk", [P, 2, 2, PAD], bf16,
                           kind="ExternalInput")
    ew_d = nc.dram_tensor("ew", [P, 512 + 16], bf16, kind="ExternalInput")
    mid_d = nc.dram_tensor("mid", [P, NM], bf16, kind="ExternalInput")
    wpl_d = nc.dram_tensor("wpl", [P, 16, 2, P], bf16, kind="ExternalInput")
    yout_d = nc.dram_tensor("yout", [2, P, J], f32, kind="ExternalOutput")

    with tile.TileContext(nc) as tc:
        with (
            tc.tile_pool(name="const", bufs=1) as cpool,
            tc.tile_pool(name="dram", bufs=1, space="DRAM") as dpool,
            tc.tile_pool(name="ppa", bufs=1, space="PSUM") as ppa,
            tc.tile_pool(name="pps", bufs=1, space="PSUM") as pps,
            tc.tile_pool(name="ppz", bufs=1, space="PSUM") as ppz,
            tc.tile_pool(name="pp4", bufs=1, space="PSUM") as pp4,
        ):
            # Touch the tensor engine ASAP: pe_busy_start persists, so an
            # early tiny matmul makes everything later run at full p-state.
            warm = cpool.tile([P, 1], f32)
            nc.gpsimd.memset(warm[:], 0.0)
            ps4 = [pp4.tile([P, J], f32, tag=f"p4{oc}",
                            name=f"ps4{oc}") for oc in range(2)]
            for w_ in range(3):
                nc.tensor.matmul(ps4[0][0:1, 0:1], warm[:, 0:1],
                                 warm[:, 0:1], start=True, stop=True)
            # ACT table (Tanh/Exp/Identity/Square set) load, before the DMAs
            nc.scalar.activation(warm[:], warm[:], AF.Tanh)
            identf = cpool.tile([P, P], f32)
            make_identity(nc, identf[:])
            identr = cpool.tile([P, P], bf16)
            nc.vector.tensor_copy(identr[:], identf[:])

            xjk = cpool.tile([P, 2, 2, PAD], bf16)
            nc.sync.dma_start(xjk[:, :, 0, :], xjk_d[:, :, 0, :])
            ew = cpool.tile([P, 512 + 16], bf16)
            nc.sync.dma_start(ew[:], ew_d[:])
            nc.sync.dma_start(xjk[:, :, 1, :], xjk_d[:, :, 1, :])
            xjg = xjk[:, 0]
            xkg = xjk[:, 1]
            pz_sb = cpool.tile([P, 9], f32)
            nc.sync.dma_start(pz_sb[:], pz_d[:])
            pvec_sb = pz_sb[:, 0:8]
            zc_sb = pz_sb[:, 8:9]
            w1_sb = ew[:, 0:512].rearrange("p (a b c) -> p a b c", a=2, b=2)
            w2_sb = ew[:, 512:528].rearrange("p (a b) -> p a b", a=2)
            mid = cpool.tile([P, NM], bf16)
            nc.sync.dma_start(mid[:], mid_d[:])
            o_ = 0
            jsel = mid[:, o_:o_ + NCH * P].rearrange(
                "p (a b) -> p a b", a=NCH)
            o_ += NCH * P
            jselt = mid[:, o_:o_ + NCH * P].rearrange(
                "p (a b) -> p a b", a=NCH)
            o_ += NCH * P
            notm = mid[:, o_:o_ + 256].rearrange("p (a b) -> p a b", a=2)
            o_ += 256
            xk_sb = mid[:, o_:o_ + 512].rearrange(
                "p (a b c) -> p a b c", a=2, b=2)
            o_ += 512
            xp_sb = mid[:, o_:o_ + NCH * 256].rearrange(
                "p (a b c) -> p a b c", a=NCH, b=2)
            o_ += NCH * 256
            xtl = mid[:, o_:o_ + 256].rearrange("p (a b) -> p a b", a=2)
            o_ += 256
            wnl = mid[:, o_:o_ + 512].rearrange(
                "p (a b c) -> p a b c", a=2, b=2)
            wpl = cpool.tile([P, 16, 2, P], bf16)
            nc.sync.dma_start(wpl[:], wpl_d[:])
            i32c = mybir.dt.int32
            magic = cpool.tile([P, 2], i32c)
            nc.vector.memset(magic[:], 0x5F3759DF)

            with tc.tile_pool(name="work", bufs=1) as wpool:
                for _rep in range(reps):
                    # four 1-bank x1 accumulators (per md,hh) so copies of
                    # one region never false-WAR matmuls into another;
                    # mm1 psums alias two of them (disjoint lifetimes).
                    px = {}
                    for md in range(2):
                        for hh in range(2):
                            px[(md, hh)] = ppa.tile(
                                [P, 4, P], f32, tag=f"px{md}{hh}",
                                name=f"px{md}{hh}")
                    ps1 = [px[(oc, 0)][:].rearrange(
                        "p a b -> p (a b)")[:, 0:PAD] for oc in range(2)]

                    # ---- phase 1: pair products -> mm1 -> tanh ----
                    ppt = wpool.tile([P, 2, PAD], bf16, tag="pp", name="ppt")
                    for dc in range(2):
                        nc.vector.tensor_mul(ppt[:, dc, :], xjg[:, dc, :],
                                             xkg[:, dc, :])
                    for dc in range(2):
                        for oc in range(2):
                            nc.tensor.matmul(ps1[oc],
                                             w1_sb[:, dc, oc, :],
                                             ppt[:, dc, :],
                                             start=(dc == 0), stop=(dc == 1))
                    # small psums: one bank for the tiny [P,8]s, one for the
                    # A-S chain (psas -> assb -> pstt -> ast is serial anyway)
                    pszl = pps.tile([P, 64], f32, tag="pszl", name="pszl")
                    psz = pszl[:, 0:H]
                    psab = ppz.tile([P, 384], f32, tag="psab", name="psab")
                    psas = psab[:, 0:256].rearrange("p (a b) -> p a b", a=2)
                    # ---- tanh per (ch, oc) so mm2 chunks start early ----
                    a_t = wpool.tile([P, 2, PAD], bf16, tag="a", name="a_t")
                    expp = wpool.tile([P, NCH, H], f32, tag="expp",
                                      name="expp")
                    expb = wpool.tile([P, NCH, H], bf16, tag="expb",
                                      name="expb")
                    for oc in range(2):
                        nc.scalar.activation(a_t[:, oc, :], ps1[oc],
                                             AF.Tanh,
                                             bias=pvec_sb[:, oc:oc + 1])
                    for ch in range(NCH):
                        cs = slice(ch * P, (ch + 1) * P)
                        psl = pszl[:, 32 + H * ch:32 + H * ch + H]
                        for oc in range(2):
                            nc.tensor.matmul(psl, a_t[:, oc, cs],
                                             w2_sb[:, oc, :],
                                             start=(oc == 0), stop=(oc == 1))
                    # A - S matmuls (PE, after mm2 in stream order)
                    for md in range(2):
                        for kc in range(2):
                            nc.tensor.matmul(psas[:, md, :],
                                             xk_sb[:, kc, md, :],
                                             notm[:, kc, :],
                                             start=(kc == 0), stop=(kc == 1))
                    nc.scalar.activation(
                        expp[:].rearrange("p a b -> p (a b)"),
                        pszl[:, 32:32 + NCH * H], AF.Exp)
                    nc.vector.tensor_copy(expb[:], expp[:])
                    assb = wpool.tile([P, 2, P], bf16, tag="assb",
                                      name="assb")
                    nc.scalar.activation(assb[:], psas, AF.Identity)

                    # AS transpose (PE) while exp finishes
                    ast = wpool.tile([P, 2, P], bf16, tag="ast", name="ast")
                    pstt = psab[:, 256:384].bitcast(bf16).rearrange(
                        "p (a b) -> p a b", a=2)
                    for md in range(2):
                        nc.tensor.transpose(pstt[:, md, :], assb[:, md, :],
                                            identr[:])

                    # ---- Z sums -> zi -> zbc broadcast (via DRAM bounce) ----
                    for ch in range(NCH):
                        nc.tensor.matmul(psz, jsel[:, ch, :],
                                         expb[:, ch, :],
                                         start=(ch == 0), stop=(ch == NCH - 1))
                    zs = wpool.tile([P, H], f32, tag="zs", name="zs")
                    nc.vector.tensor_scalar_add(out=zs[:], in0=psz,
                                                scalar1=zc_sb)
                    zi = wpool.tile([P, H], f32, tag="zi", name="zi")
                    nc.vector.reciprocal(zi[:], zs[:])
                    zib = wpool.tile([P, H], bf16, tag="zib", name="zib")
                    nc.vector.tensor_copy(zib[:], zi[:])

                    # ---- phase 3: zin gather -> en -> jh builds ----
                    # jh[p,(h,j)] = exp[p,h]*zi[row(p),h]*JSEL[p,j]; pseudo
                    # chunk diag(zi_h) carries the masked-mass (A-S) term.
                    jh = wpool.tile([P, NCH, H, P], bf16, tag="jh", name="jh")
                    for ch in range(NCH):
                        pszn = pszl[:, H + H * ch:H + H * ch + H]
                        nc.tensor.matmul(pszn, jselt[:, ch, :], zib[:],
                                         start=True, stop=True)
                        en = wpool.tile([P, H], f32, tag="en", bufs=3,
                                        name=f"en_{ch}")
                        nc.vector.tensor_mul(en[:], expp[:, ch, :], pszn)
                        for h in range(H):
                            eng = nc.gpsimd if h >= 6 else nc.vector
                            eng.tensor_scalar_mul(out=jh[:, ch, h, :],
                                                  in0=jsel[:, ch, :],
                                                  scalar1=en[:, h:h + 1])
                        if ch == 0:
                            # ast copies here: off the en->jh critical path
                            for md in range(2):
                                nc.vector.tensor_copy(ast[:, md, :],
                                                      pstt[:, md, :])
                    jhps = wpool.tile([P, H, P], bf16, tag="jhps",
                                      name="jhps")
                    for h in range(H):
                        eng = nc.gpsimd if h >= 5 else nc.vector
                        eng.tensor_scalar_mul(out=jhps[:, h, :],
                                              in0=identr[:],
                                              scalar1=zi[:, h:h + 1])

                    # ---- phase 4: x1 accumulation, split copies ----
                    x1sb = wpool.tile([P, 2, H, P], bf16, tag="x1sb",
                                      name="x1sb")
                    for md in range(2):
                        for hh in range(2):
                            hs = slice(hh * 4, hh * 4 + 4)
                            pxt = px[(md, hh)]
                            out = pxt[:].rearrange("p a b -> p (a b)")
                            for ch in range(NCH):
                                nc.tensor.matmul(
                                    out, xp_sb[:, ch, md, :],
                                    jh[:, ch, hs, :].rearrange(
                                        "p a b -> p (a b)"),
                                    start=(ch == 0), stop=False)
                            nc.tensor.matmul(
                                out, ast[:, md, :],
                                jhps[:, hs, :].rearrange("p a b -> p (a b)"),
                                start=False, stop=True)
                            if hh == 0:
                                nc.scalar.activation(x1sb[:, md, hs, :],
                                                     pxt[:], AF.Identity)
                            else:
                                nc.vector.tensor_copy(x1sb[:, md, hs, :],
                                                      pxt[:])

                    # ---- phase 5: output projection (split ps4 tiles) ----
                    stats = wpool.tile([P, 4], f32, tag="stats", name="stats")
                    for oc in range(2):
                        first = True
                        for md in range(2):
                            for h in range(H):
                                nc.tensor.matmul(ps4[oc][:],
                                                 wpl[:, h * 2 + md, oc, :],
                                                 x1sb[:, md, h, :],
                                                 start=first, stop=False)
                                first = False
                        for dc in range(2):
                            nc.tensor.matmul(ps4[oc][:], wnl[:, dc, oc, :],
                                             xtl[:, dc, :],
                                             start=False, stop=(dc == 1))
                    # y (with bias) into one [P,2,J] tile; merged DVE stats:
                    # s1 pair and s2 pair each one reduce -> two pipelined
                    # [P,2] all-reduces (s1s then s2s)
                    statg = wpool.tile([P, 2, 2], f32, tag="statg",
                                       name="statg")
                    AX = mybir.AxisListType.X
                    yb = wpool.tile([P, 2, J], f32, tag="yb", name="yb")
                    y_t = [yb[:, oc, :] for oc in range(2)]
                    sq = wpool.tile([P, 2, J], f32, tag="sq", name="sqt")
                    ccs = []
                    for oc in range(2):
                        nc.scalar.activation(yb[:, oc, :], ps4[oc][:],
                                             AF.Identity,
                                             bias=pvec_sb[:, 2 + oc:3 + oc])
                        nc.vector.tensor_reduce(
                            stats[:, 2 * oc:2 * oc + 1], yb[:, oc, :],
                            axis=AX, op=ALU.add)
                        nc.gpsimd.tensor_mul(sq[:, oc, :], yb[:, oc, :],
                                             yb[:, oc, :])
                        nc.vector.tensor_reduce(
                            stats[:, 2 * oc + 1:2 * oc + 2], sq[:, oc, :],
                            axis=AX, op=ALU.add)
                        cc_in = dpool.tile([P, 2], f32, name=f"cc_in{oc}")
                        cc_out = dpool.tile([P, 2], f32, addr_space="Shared",
                                            name=f"cc_out{oc}")
                        ccs.append((cc_in, cc_out))
                        nc.sync.dma_start(cc_in[:],
                                          stats[:, 2 * oc:2 * oc + 2])
                    for oc in range(2):
                        cc_in, cc_out = ccs[oc]
                        if with_collective:
                            nc.gpsimd.collective_compute(
                                "AllReduce",
                                ALU.add,
                                replica_groups=[list(range(NCORES))],
                                ins=[cc_in.opt()],
                                outs=[cc_out.opt()],
                            )
                        else:
                            nc.sync.dma_start(cc_out[:], cc_in[:])
                    for oc in range(2):
                        # statg[:, s, oc] <- this oc's (s1, s2), strided
                        nc.sync.dma_start(statg[:, :, oc], ccs[oc][1][:])

                    NTOT = float(B * T)

                    def wt2(nm):
                        return wpool.tile([P, 2], f32, tag=nm, name=nm)

                    mom = wpool.tile([P, 4], f32, tag="mom", name="mom")
                    nc.vector.tensor_scalar_mul(out=mom[:, 0:2],
                                                in0=statg[:, 0, :],
                                                scalar1=1.0 / NTOT)
                    nc.vector.tensor_scalar(out=mom[:, 2:4],
                                            in0=statg[:, 1, :],
                                            scalar1=1.0 / NTOT,
                                            scalar2=BN_EPS,
                                            op0=ALU.mult, op1=ALU.add)
                    mu = mom[:, 0:2]
                    varp = mom[:, 2:4]
                    musq = wt2("musq")
                    nc.vector.tensor_mul(musq[:], mu, mu)
                    nc.vector.tensor_sub(varp, varp, musq[:])
                    i32 = mybir.dt.int32
                    ri = wpool.tile([P, 2], i32, tag="ri", name="ri")
                    nc.vector.tensor_scalar(out=ri[:], in0=varp.bitcast(i32),
                                            scalar1=1, scalar2=None,
                                            op0=ALU.arith_shift_right)
                    rstd = wt2("rstd")
                    nc.vector.tensor_sub(rstd[:].bitcast(i32), magic[:],
                                         ri[:])
                    ra = wt2("ra")
                    rb = wt2("rb")
                    for _ in range(1):
                        nc.vector.tensor_mul(ra[:], rstd[:], rstd[:])
                        nc.vector.scalar_tensor_tensor(
                            out=rb[:], in0=ra[:], scalar=-0.5, in1=varp,
                            op0=ALU.mult, op1=ALU.mult)
                        nc.vector.tensor_scalar_add(out=rb[:], in0=rb[:],
                                                    scalar1=1.5)
                        nc.vector.tensor_mul(rstd[:], rstd[:], rb[:])
                    scl = wt2("scl")
                    nc.vector.tensor_mul(scl[:], pvec_sb[:, 4:6], rstd[:])
                    tmp = wt2("tmp")
                    nc.vector.tensor_mul(tmp[:], mu, scl[:])
                    shf = wt2("shf")
                    nc.vector.tensor_sub(shf[:], pvec_sb[:, 6:8], tmp[:])

                    z = wpool.tile([P, 2, J], f32, tag="z", name="z")
                    for oc in range(2):
                        nc.vector.tensor_scalar(out=z[:, oc, :],
                                                in0=y_t[oc][:],
                                                scalar1=scl[:, oc:oc + 1],
                                                scalar2=shf[:, oc:oc + 1],
                                                op0=ALU.mult, op1=ALU.add)
                    # selu: lam*max(z,0) - lam*alpha + lam*alpha*e^min(z,0)
                    neg = wpool.tile([P, 2, J], f32, tag="neg", name="neg")
                    nc.gpsimd.tensor_scalar_min(out=neg[:], in0=z[:],
                                                scalar1=0.0)
                    ep = wpool.tile([P, 2, J], f32, tag="ep", name="ep")
                    nc.scalar.activation(ep[:], neg[:], AF.Exp)
                    pos = wpool.tile([P, 2, J], f32, tag="pos", name="pos")
                    nc.vector.tensor_scalar(
                        out=pos[:], in0=z[:], scalar1=0.0,
                        scalar2=SELU_LAM,
                        op0=ALU.max, op1=ALU.mult)
                    nc.vector.tensor_scalar_add(
                        out=pos[:], in0=pos[:],
                        scalar1=-SELU_LAM * SELU_ALPHA)
                    outz = wpool.tile([P, 2, J], f32, tag="outz", name="outz")
                    nc.vector.scalar_tensor_tensor(
                        out=outz[:], in0=ep[:], scalar=SELU_LAM * SELU_ALPHA,
                        in1=pos[:], op0=ALU.mult, op1=ALU.add)
                    nc.sync.dma_start(yout_d.ap().rearrange("c p j -> p c j"),
                                      outz[:])

    nc.compile()
    return nc


def _prep_inputs(x, boundary, att_proj_w, att_proj_b, att_weight,
                 proj_att_w, proj_att_b, proj_no_w, proj_no_b,
                 bn_gamma, bn_beta):
    x = np.ascontiguousarray(np.asarray(x, dtype=np.float32))
    bnd = np.asarray(boundary)
    mask = _message_control_mask_np(bnd)
    W1 = np.asarray(att_proj_w, np.float32)
    W2 = np.asarray(att_weight, np.float32)
    Wp = np.asarray(proj_att_w, np.float32)
    Wn = np.asarray(proj_no_w, np.float32)

    by = (np.asarray(proj_att_b, np.float32)
          + np.asarray(proj_no_b, np.float32))
    b1 = np.asarray(att_proj_b, np.float32)
    g = np.asarray(bn_gamma, np.float32)
    be = np.asarray(bn_beta, np.float32)
    pvec = np.zeros((P, 8), dtype=np.float32)
    for oc in range(2):
        pvec[:, oc] = b1[oc * P:(oc + 1) * P]
        pvec[:, 2 + oc] = by[oc * P:(oc + 1) * P]
        pvec[:, 4 + oc] = g[oc * P:(oc + 1) * P]
        pvec[:, 6 + oc] = be[oc * P:(oc + 1) * P]

    # shared weight layouts
    W1l = np.zeros((P, 2, 2, P), np.float32)
    WNl = np.zeros((P, 2, 2, P), np.float32)
    for dc in range(2):
        for oc in range(2):
            W1l[:, dc, oc, :] = W1[dc * P:(dc + 1) * P, oc * P:(oc + 1) * P]
            WNl[:, dc, oc, :] = Wn[dc * P:(dc + 1) * P, oc * P:(oc + 1) * P]
    W2c = np.zeros((P, 2, H), np.float32)
    for oc in range(2):
        W2c[:, oc, :] = W2[oc * P:(oc + 1) * P, :]
    Wp3 = Wp.reshape(D, H, O)  # (d, h, o)
    WPl = np.zeros((P, 16, 2, P), np.float32)
    for h in range(H):
        for md in range(2):
            c16 = h * 2 + md
            for oc in range(2):
                WPl[:, c16, oc, :] = Wp3[md * P:(md + 1) * P, h,
                                         oc * P:(oc + 1) * P]

    segs = [_seg_of(bnd[bb]) for bb in range(B)]
    # uniform chunk count across cores (SPMD)
    phat = []
    for c in range(NCORES):
        bb, j0 = c // 2, (c % 2) * J
        phat.append(int(segs[bb][j0:j0 + J, 1].sum()))
    nch = max(1, int(np.ceil(max(phat) / P)))
    _CACHE["nch"] = nch
    PAD = nch * P

    in_maps = []
    for c in range(NCORES):
        bb, j0 = c // 2, (c % 2) * J
        xb = x[bb]  # (T, D)
        seg = segs[bb]
        rows = []   # jl per pair
        keys = []   # global k per pair
        zc = np.zeros((P, 1), np.float32)
        for jl in range(J):
            k0, L = seg[j0 + jl]
            rows += [jl] * int(L)
            keys += list(range(int(k0), int(k0 + L)))
            zc[jl, 0] = float(T - L)
        np_pairs = len(rows)
        assert np_pairs <= PAD
        rows = np.asarray(rows + [0] * (PAD - np_pairs), np.int64)
        keys = np.asarray(keys + [0] * (PAD - np_pairs), np.int64)
        valid = np.zeros(PAD, np.float32)
        valid[:np_pairs] = 1.0

        xT = xb.T  # (D, T)
        XJg = np.zeros((P, 2, PAD), np.float32)
        XKg = np.zeros((P, 2, PAD), np.float32)
        Xp = np.zeros((P, nch, 2, P), np.float32)
        for dc in range(2):
            XJg[:, dc, :] = xT[dc * P:(dc + 1) * P, j0 + rows] * valid
            XKg[:, dc, :] = xT[dc * P:(dc + 1) * P, keys] * valid
        xpk = xb[keys] * valid[:, None]  # (PAD, D)
        for ch in range(nch):
            for md in range(2):
                Xp[:, ch, md, :] = xpk[ch * P:(ch + 1) * P,
                                       md * P:(md + 1) * P]
        JSEL = np.zeros((P, nch, P), np.float32)
        JSELT = np.zeros((P, nch, P), np.float32)
        for p in range(np_pairs):
            ch, pp = p // P, p % P
            JSEL[pp, ch, rows[p]] = 1.0
            JSELT[rows[p], ch, pp] = 1.0
        mrow = mask[bb, j0:j0 + J, :]  # (J, T)
        NOTM = np.zeros((P, 2, P), np.float32)
        for kc in range(2):
            NOTM[:, kc, :] = 1.0 - mrow[:, kc * P:(kc + 1) * P].T
        xkl = np.zeros((P, 2, 2, P), np.float32)
        for kc in range(2):
            for md in range(2):
                xkl[:, kc, md, :] = xb[kc * P:(kc + 1) * P,
                                       md * P:(md + 1) * P]
        xtl = np.zeros((P, 2, P), np.float32)
        for dc in range(2):
            xtl[:, dc, :] = xT[dc * P:(dc + 1) * P, j0:j0 + J]

        pzv = np.concatenate([pvec, zc], axis=1)  # (P, 9)
        ew = np.concatenate([
            W1l.reshape(P, -1), W2c.reshape(P, -1)], axis=1)
        mid = np.concatenate([
            JSEL.reshape(P, -1), JSELT.reshape(P, -1),
            NOTM.reshape(P, -1), xkl.reshape(P, -1),
            Xp.reshape(P, -1), xtl.reshape(P, -1),
            WNl.reshape(P, -1)], axis=1)
        in_maps.append({
            "pz": pzv.astype(np.float32),
            "xjk": np.stack([XJg, XKg], axis=1).astype(BF),
            "ew": ew.astype(BF),
            "mid": mid.astype(BF),
            "wpl": WPl.astype(BF),
        })
    return in_maps


def kernel(**inputs):
    from concourse.bass_utils import run_bass_kernel_spmd

    in_maps = _prep_inputs(**inputs)
    nch = _CACHE["nch"]
    key = ("nc", nch)
    if key not in _CACHE:
        _CACHE[key] = _build_module(nch=nch)
    nc = _CACHE[key]

    res = run_bass_kernel_spmd(nc, in_maps, core_ids=list(range(NCORES)),
                               **_CACHE.get("run_kwargs", {}))
    _CACHE["last_results"] = res

    out = np.zeros((B, T, O), dtype=np.float32)
    for c in range(NCORES):
        bb, j0 = c // 2, (c % 2) * J
        yc = res.results[c]["yout"]  # (2, P, J)
        out[bb, j0:j0 + J, :] = yc.reshape(O, J).T
    return out


if __name__ == "__main__":
    _build_module(nch=3)
    print("build ok")
